# revision 58
# baseline (speedup 1.0000x reference)
"""Trainium2 Bass kernel for the double-FIR "DeconvLayer" problem.

Reference computation (see problem statement):
    v = mask(conv(x, k1)),  y = mask(conv(v, k2))
with k1 = [1, h], k2 = [1, h_reversed], mask zeroing columns < 16.

Key facts used here:
  * For output column t >= 48 the two masked passes are EXACTLY one causal
    33-tap conv with kc = full_conv(k1, k2):  y[t] = sum_d kc[d] x[t-d].
  * kc[0] = 1, so y = x + e with e = sum_{d>=1} kc[d] x[t-d].  The tail
    term e is small (std ~0.19 vs y std ~1.02), so the DEVICE computes e
    from an fp8 input stream and returns e as fp8; the host assembly adds
    the exactly-kept fp32 x back in.  Wire traffic drops 2x vs fp16 while
    the total rel-l2 error stays ~9e-3 (gate is 2e-2).
  * y[:, 0:128] depends only on x[:, 0:128]: handled exactly (up to fp8)
    by a dense matmul with M0' = M0 - I on separately-staged copies of
    each row's block 0.

Device strategy (pure data parallel, 32 batch rows / core, 8 cores):
  * Host marshals x into time-major columns: each column of the device
    input is one aligned 128-sample block of one row.  Then
        e_block(c) = A.T @ x_block(c) + C.T @ x_block(c-1)
    with A[p, po] = kc[po - p] (in-block taps, d >= 1 only) and
    C[p, po] = kc[po + 128 - p] (halo taps from the previous block).
  * fp8 (e4m3) everywhere on the wire and in the matmuls.  The two
    128-deep contractions (A on block c, C on block c-1) fuse into ONE
    K=256 matmul via MatmulPerfMode.DoubleRow: the stationary operand is
    [128, 2, 128] = [C | A], the moving operand is an overlapping-stride
    AP [128, 2, 512] whose k-tiles are the halo-shifted and in-block
    column windows of the same SBUF data.  One matmul per PSUM tile, one
    weight load for the whole kernel.
  * Engines: DMA (in/out), PE (DoubleRow matmuls), DVE+ACT (PSUM->SBUF
    fp8 casts).  Per-core wire: ~4.2 MB in + ~4.2 MB out, vs the
    ~358 GB/s per-core HBM cap -> ~23.5 us DMA floor.

The harness calls kernel(**inputs) with the FULL inputs; everything the
device needs (shapes, tiling) is hardcoded below.
"""

import numpy as np
import ml_dtypes

import concourse.bass as bass
import concourse.mybir as mybir
from concourse import bacc
from concourse.bass_utils import run_bass_kernel_spmd
from concourse.tile import TileContext

# ---------------------------------------------------------------- geometry
B, N, F = 256, 131072, 16
N_CORES = 8
RPC = B // N_CORES          # 32 batch rows per core
BLK = 128                   # time block = matmul contraction window
BPR = N // BLK              # 1024 blocks per row
DCOLS = RPC * BPR           # 32768 data columns per core
MT = 4096                   # macro-tile: columns per in/out DMA
NMT = DCOLS // MT           # 8 macro tiles per core
PT = 512                    # psum tile columns (one fp32 PSUM bank)
W_IN = MT + 16              # input cols per macro tile (halo + align pad)
XCOLS = NMT * W_IN + RPC    # device input width (+32 row-initial blocks)
YPAD = 32                   # dead lead-in keeps out-DMA rows 32B-aligned
YCOLS = YPAD + DCOLS + RPC  # device output: pad, main, first-block tail

_F8 = mybir.dt.float8e4
_F32 = mybir.dt.float32
_NP8 = ml_dtypes.float8_e4m3

# ------------------------------------------------------------- bass program
_NC_CACHE = None


def _build_nc():
    nc = bacc.Bacc()
    xp = nc.dram_tensor("xp", [128, XCOLS], _F8, kind="ExternalInput")
    wm = nc.dram_tensor("wm", [128, 384], _F8, kind="ExternalInput")
    yT = nc.dram_tensor("yT", [128, YCOLS], _F8, kind="ExternalOutput")

    with TileContext(nc) as tc:
        with (
            tc.tile_pool(name="w", bufs=1) as wpool,
            tc.tile_pool(name="xin", bufs=5) as xpool,
            tc.tile_pool(name="stage", bufs=6) as spool,
            tc.tile_pool(name="ps", bufs=4, space="PSUM") as pspool,
        ):
            # Weights ride Sync FIRST (49 KB, gates the first LDWEIGHTS),
            # then the input stream owns the queue.  Tile 0 leads with a
            # small chunk so the first matmul starts as early as possible.
            wsb = wpool.tile([128, 384], _F8, tag="wsb")
            nc.sync.dma_start(out=wsb[:], in_=wm[:, :])

            QC = 544   # covers PSUM tile 0 (15 + 512 + halo) + slack
            HC = 2064  # chunk boundary aligned to pair-1's window end
            xt0 = xpool.tile([128, W_IN], _F8, tag="xt")
            nc.sync.dma_start(out=xt0[:, 0:QC], in_=xp[:, 0:QC])
            nc.sync.dma_start(out=xt0[:, QC:HC], in_=xp[:, QC:HC])

            # NOTE: splitting the input stream onto a second DMA path was
            # measured a net loss in every dose: ACT-issued input DMAs
            # wedge the device outright, and SWDGE (gpsimd) input steals
            # HBM bandwidth from the critical early tiles.  Input stays
            # exclusively on the Sync HWDGE ring.
            # DoubleRow stationary operand: ktile0 = C, ktile1 = A.
            w3 = bass.AP(
                tensor=wsb[:].tensor,
                offset=wsb[:].offset,
                ap=[[384, 128], [128, 2], [1, 128]],
            )
            M0 = wsb[:, 256:384]

            # Row-initial blocks for the dense M0' map; prefetched early,
            # consumed by the LAST matmul (see below).
            et = xpool.tile([128, RPC], _F8, tag="extra")
            nc.gpsimd.dma_start(out=et[:], in_=xp[:, NMT * W_IN : XCOLS])

            for m in range(NMT):
                if m == 0:
                    xt = xt0
                    nc.sync.dma_start(
                        out=xt[:, HC:W_IN], in_=xp[:, HC:W_IN]
                    )
                else:
                    xt = xpool.tile([128, W_IN], _F8, tag="xt")
                    w0 = m * W_IN
                    if m <= 3:
                        # Ramp-critical tiles ride in halves: each tile's
                        # first pairs unlock one ~1.9us DMA receipt
                        # earlier.  Steady-state tiles stay whole for DMA
                        # efficiency.
                        nc.sync.dma_start(
                            out=xt[:, 0:HC], in_=xp[:, w0 : w0 + HC]
                        )
                        nc.sync.dma_start(
                            out=xt[:, HC:W_IN],
                            in_=xp[:, w0 + HC : w0 + W_IN],
                        )
                    else:
                        nc.sync.dma_start(
                            out=xt[:], in_=xp[:, w0 : w0 + W_IN]
                        )
                stage = spool.tile([128, MT], _F8, tag="stage")

                # One fused K=256 DoubleRow matmul per 512-col PSUM tile:
                # ktile0 reads the halo-shifted window (col 15+t0+j), ktile1
                # the in-block window (col 16+t0+j) of the same xt bytes.
                # Drain granularity: 1024-col drains are ~25% cheaper per
                # column (fixed instruction overhead), giving DVE/ACT
                # headroom over the PE rate; the LAST tile uses 512-col
                # drains + immediate sync-ring DMAs so the epilogue isn't
                # gated on a coarse drain.
                last = m == NMT - 1
                for pr in range(MT // (2 * PT)):
                    pp = pspool.tile(
                        [128, 2 * PT], _F32, name=f"pp_{m}_{pr}", tag="ps"
                    )
                    for half in range(2):
                        t0 = (2 * pr + half) * PT
                        x3 = bass.AP(
                            tensor=xt[:].tensor,
                            offset=xt[:].offset + 15 + t0,
                            ap=[[W_IN, 128], [1, 2], [1, PT]],
                        )
                        nc.tensor.matmul(
                            pp[:, half * PT : (half + 1) * PT], w3, x3,
                            start=True, stop=True,
                            perf_mode=mybir.MatmulPerfMode.DoubleRow,
                        )
                    t0 = 2 * pr * PT
                    if not last:
                        # One 1024-col drain per pair (better per-column
                        # rate than 2x512), strictly alternating DVE/ACT.
                        # Re-balancing drains toward the faster ACT engine
                        # was measured WORSE in any placement - the clean
                        # alternating cadence pipelines better than a
                        # nominally balanced split.
                        if pr % 2 == 0:
                            nc.vector.tensor_copy(
                                out=stage[:, t0 : t0 + 2 * PT], in_=pp[:]
                            )
                        else:
                            nc.scalar.copy(
                                out=stage[:, t0 : t0 + 2 * PT], in_=pp[:]
                            )
                        if pr == MT // (2 * PT) - 1:
                            # One whole-tile out-DMA on the GpSimd SWDGE
                            # queue (Q7 emission is ~1us each, so fewer
                            # bigger DMAs; the 6-deep stage ring hides
                            # their completion latency).
                            nc.gpsimd.dma_start(
                                out=yT[
                                    :, YPAD + m * MT : YPAD + (m + 1) * MT
                                ],
                                in_=stage[:],
                            )
                    else:
                        # Last tile: split the pair drain across BOTH
                        # engines and ship each 2048-col half on its own
                        # queue (gpsimd then sync) -> shortest epilogue.
                        nc.vector.tensor_copy(
                            out=stage[:, t0 : t0 + PT], in_=pp[:, 0:PT]
                        )
                        nc.scalar.copy(
                            out=stage[:, t0 + PT : t0 + 2 * PT],
                            in_=pp[:, PT : 2 * PT],
                        )
                        nc.sync.dma_start(
                            out=yT[
                                :,
                                YPAD + m * MT + t0 : YPAD + m * MT + t0
                                + 2 * PT,
                            ],
                            in_=stage[:, t0 : t0 + 2 * PT],
                        )

            # First 128 samples of every row via the dense M0' map.  This
            # plain matmul MUST come after every DoubleRow matmul: a
            # DoubleRow matmul as the final PE instruction wedges the PE
            # (NRT_EXEC_UNIT_UNRECOVERABLE, found by bisection).  Its
            # out-DMA rides the idle Sync queue to keep the epilogue short.
            ps2 = pspool.tile([128, RPC], _F32, name="ps2", tag="ps")
            nc.tensor.matmul(ps2[:], M0, et[:], start=True, stop=True)
            st2 = spool.tile([128, RPC], _F8, tag="st2")
            nc.vector.tensor_copy(out=st2[:], in_=ps2[:])
            nc.sync.dma_start(out=yT[:, YPAD + DCOLS : YCOLS], in_=st2[:])

    nc.compile()  # bacc legalization: <=1 sync wait per HW instruction
    return nc


def _get_nc():
    global _NC_CACHE
    if _NC_CACHE is None:
        _NC_CACHE = _build_nc()
    return _NC_CACHE


# ------------------------------------------------------------- host helpers
def _fir_mat(taps):
    """128x128 matrix of one masked FIR pass: y = T @ x (first 128 samples).

    y[i] = x[i] + sum_j taps[j] * x[i-j-1] for i >= F, else 0.
    """
    T = np.zeros((128, 128))
    for i in range(F, 128):
        T[i, i] = 1.0
        for j in range(F):
            T[i, i - j - 1] += taps[j]
    return T


def _build_weights(h64):
    """Stationary operands, stacked as [C A M0'], fp8."""
    k1 = np.concatenate([[1.0], h64])
    k2 = np.concatenate([[1.0], h64[::-1]])
    kc = np.convolve(k1, k2)  # 33 taps

    i = np.arange(128)
    D = i[None, :] - i[:, None]  # po - p
    A = np.zeros((128, 128))
    mask = (D >= 1) & (D <= 32)  # tail taps only; identity handled on host
    A[mask] = kc[D[mask]]
    Dc = D + 128
    C = np.zeros((128, 128))
    maskc = (Dc >= 1) & (Dc <= 32)
    C[maskc] = kc[Dc[maskc]]

    M0 = (_fir_mat(h64[::-1]) @ _fir_mat(h64)).T  # M0[p, po] = dy[po]/dx[p]
    M0 -= np.eye(128)  # tail-only: host adds x back everywhere

    wmat = np.zeros((128, 384), np.float32)
    wmat[:, 0:128] = C
    wmat[:, 128:256] = A
    wmat[:, 256:384] = M0
    return wmat.astype(_NP8)


def _make_in_maps(x, h64):
    wmat = _build_weights(h64)
    in_maps = []
    for c in range(N_CORES):
        xs = np.ascontiguousarray(x[c * RPC : (c + 1) * RPC])  # (32, 131072)
        Bv = xs.reshape(RPC, BPR, BLK)
        xin = np.zeros((128, 1 + DCOLS), _NP8)
        xin[:, 1:] = Bv.transpose(2, 0, 1).reshape(BLK, DCOLS).astype(_NP8)
        xpk = np.zeros((128, XCOLS), _NP8)
        for m in range(NMT):
            # halo col at slot 15, current cols at slots 16..16+MT-1
            xpk[:, m * W_IN + 15 : (m + 1) * W_IN] = (
                xin[:, m * MT : m * MT + MT + 1]
            )
        xpk[:, NMT * W_IN : XCOLS] = Bv[:, 0, :].astype(_NP8).T
        in_maps.append({"xp": xpk, "wm": wmat})
    return in_maps


def _assemble(x, results):
    y = np.empty((B, N), np.float32)
    for c in range(N_CORES):
        yT = results[c]["yT"].astype(np.float32)
        main = (
            yT[:, YPAD : YPAD + DCOLS]
            .reshape(BLK, RPC, BPR)
            .transpose(1, 2, 0)
            .reshape(RPC, N)
        )
        sl = slice(c * RPC, (c + 1) * RPC)
        y[sl] = x[sl] + main
        y[sl, 0:BLK] = x[sl, 0:BLK] + yT[:, YPAD + DCOLS : YCOLS].T
    y[:, :F] = 0.0
    return y


def _run(x, h, **spmd_kwargs):
    x = np.asarray(x, dtype=np.float32)
    h64 = np.asarray(h, dtype=np.float64).reshape(-1)
    assert x.shape == (B, N) and h64.shape == (F,)
    in_maps = _make_in_maps(x, h64)
    res = run_bass_kernel_spmd(
        _get_nc(), in_maps, core_ids=list(range(N_CORES)), **spmd_kwargs
    )
    return _assemble(x, res.results), res


def kernel(x, kernel):
    y, _ = _run(x, kernel)
    return y


# revision 60
# speedup vs baseline: 1.0585x; 1.0585x over previous
"""Trainium2 Bass kernel for the double-FIR "DeconvLayer" problem.

Reference computation (see problem statement):
    v = mask(conv(x, k1)),  y = mask(conv(v, k2))
with k1 = [1, h], k2 = [1, h_reversed], mask zeroing columns < 16.

Key facts used here:
  * For output column t >= 48 the two masked passes are EXACTLY one causal
    33-tap conv with kc = full_conv(k1, k2):  y[t] = sum_d kc[d] x[t-d].
  * kc[0] = 1, so y = x + e with e = sum_{d>=1} kc[d] x[t-d].  The tail
    term e is small (std ~0.19 vs y std ~1.02), so the DEVICE computes e
    from an fp8 input stream and returns e as fp8; the host assembly adds
    the exactly-kept fp32 x back in.  Wire traffic drops 2x vs fp16 while
    the total rel-l2 error stays ~9e-3 (gate is 2e-2).
  * y[:, 0:128] depends only on x[:, 0:128]: handled exactly (up to fp8)
    by a dense matmul with M0' = M0 - I on separately-staged copies of
    each row's block 0.

Device strategy (pure data parallel, 32 batch rows / core, 8 cores):
  * Host marshals x into time-major columns: each column of the device
    input is one aligned 128-sample block of one row.  Then
        e_block(c) = A.T @ x_block(c) + C.T @ x_block(c-1)
    with A[p, po] = kc[po - p] (in-block taps, d >= 1 only) and
    C[p, po] = kc[po + 128 - p] (halo taps from the previous block).
  * fp8 (e4m3) everywhere on the wire and in the matmuls.  The two
    128-deep contractions (A on block c, C on block c-1) fuse into ONE
    K=256 matmul via MatmulPerfMode.DoubleRow: the stationary operand is
    [128, 2, 128] = [C | A], the moving operand is an overlapping-stride
    AP [128, 2, 512] whose k-tiles are the halo-shifted and in-block
    column windows of the same SBUF data.  One matmul per PSUM tile, one
    weight load for the whole kernel.
  * Engines: DMA (in/out), PE (DoubleRow matmuls), DVE+ACT (PSUM->SBUF
    fp8 casts).  Per-core wire: ~4.2 MB in + ~4.2 MB out, vs the
    ~358 GB/s per-core HBM cap -> ~23.5 us DMA floor.

The harness calls kernel(**inputs) with the FULL inputs; everything the
device needs (shapes, tiling) is hardcoded below.
"""

import numpy as np
import ml_dtypes

import concourse.bass as bass
import concourse.mybir as mybir
from concourse import bacc
from concourse.bass_utils import run_bass_kernel_spmd
from concourse.tile import TileContext

# ---------------------------------------------------------------- geometry
B, N, F = 256, 131072, 16
N_CORES = 8
RPC = B // N_CORES          # 32 batch rows per core
BLK = 128                   # time block = matmul contraction window
BPR = N // BLK              # 1024 blocks per row
DCOLS = RPC * BPR           # 32768 data columns per core
MT = 4096                   # macro-tile: columns per in/out DMA
NMT = DCOLS // MT           # 8 macro tiles per core
PT = 512                    # psum tile columns (one fp32 PSUM bank)
W_IN = MT + 16              # input cols per macro tile (halo + align pad)
XCOLS = NMT * W_IN + RPC    # device input width (+32 row-initial blocks)
YPAD = 32                   # dead lead-in keeps out-DMA rows 32B-aligned
YCOLS = YPAD + DCOLS + RPC  # device output: pad, main, first-block tail

_F8 = mybir.dt.float8e4
_F32 = mybir.dt.float32
_NP8 = ml_dtypes.float8_e4m3

# ------------------------------------------------------------- bass program
_NC_CACHE = None


def _build_nc():
    nc = bacc.Bacc()
    xp = nc.dram_tensor("xp", [128, XCOLS], _F8, kind="ExternalInput")
    wm = nc.dram_tensor("wm", [128, 384], _F8, kind="ExternalInput")
    yT = nc.dram_tensor("yT", [128, YCOLS], _F8, kind="ExternalOutput")

    with TileContext(nc) as tc:
        with (
            tc.tile_pool(name="w", bufs=1) as wpool,
            tc.tile_pool(name="xin", bufs=5) as xpool,
            tc.tile_pool(name="stage", bufs=6) as spool,
            tc.tile_pool(name="ps", bufs=4, space="PSUM") as pspool,
        ):
            # Weights ride Sync FIRST (49 KB, gates the first LDWEIGHTS),
            # then the input stream owns the queue.  Tile 0 leads with a
            # small chunk so the first matmul starts as early as possible.
            wsb = wpool.tile([128, 384], _F8, tag="wsb")
            nc.sync.dma_start(out=wsb[:], in_=wm[:, :])

            QC = 544   # covers PSUM tile 0 (15 + 512 + halo) + slack
            HC = 2064  # chunk boundary aligned to pair-1's window end
            xt0 = xpool.tile([128, W_IN], _F8, tag="xt")
            nc.sync.dma_start(out=xt0[:, 0:QC], in_=xp[:, 0:QC])
            nc.sync.dma_start(out=xt0[:, QC:HC], in_=xp[:, QC:HC])

            # NOTE: splitting the input stream onto a second DMA path was
            # measured a net loss in every dose: ACT-issued input DMAs
            # wedge the device outright, and SWDGE (gpsimd) input steals
            # HBM bandwidth from the critical early tiles.  Input stays
            # exclusively on the Sync HWDGE ring.
            # DoubleRow stationary operand: ktile0 = C, ktile1 = A.
            w3 = bass.AP(
                tensor=wsb[:].tensor,
                offset=wsb[:].offset,
                ap=[[384, 128], [128, 2], [1, 128]],
            )
            M0 = wsb[:, 256:384]

            # Row-initial blocks for the dense M0' map; prefetched early,
            # consumed by the LAST matmul (see below).
            et = xpool.tile([128, RPC], _F8, tag="extra")
            nc.gpsimd.dma_start(out=et[:], in_=xp[:, NMT * W_IN : XCOLS])

            for m in range(NMT):
                if m == 0:
                    xt = xt0
                    nc.sync.dma_start(
                        out=xt[:, HC:W_IN], in_=xp[:, HC:W_IN]
                    )
                else:
                    # Whole-tile input DMAs: half-splitting the early
                    # tiles (to unlock their first pairs a DMA receipt
                    # earlier) measured neutral-to-worse - the extra
                    # issues occasionally disturb the ramp.
                    xt = xpool.tile([128, W_IN], _F8, tag="xt")
                    w0 = m * W_IN
                    nc.sync.dma_start(
                        out=xt[:], in_=xp[:, w0 : w0 + W_IN]
                    )
                stage = spool.tile([128, MT], _F8, tag="stage")

                # One fused K=256 DoubleRow matmul per 512-col PSUM tile:
                # ktile0 reads the halo-shifted window (col 15+t0+j), ktile1
                # the in-block window (col 16+t0+j) of the same xt bytes.
                # Drain granularity: 1024-col drains are ~25% cheaper per
                # column (fixed instruction overhead), giving DVE/ACT
                # headroom over the PE rate; the LAST tile uses 512-col
                # drains + immediate sync-ring DMAs so the epilogue isn't
                # gated on a coarse drain.
                last = m == NMT - 1
                for pr in range(MT // (2 * PT)):
                    pp = pspool.tile(
                        [128, 2 * PT], _F32, name=f"pp_{m}_{pr}", tag="ps"
                    )
                    for half in range(2):
                        t0 = (2 * pr + half) * PT
                        x3 = bass.AP(
                            tensor=xt[:].tensor,
                            offset=xt[:].offset + 15 + t0,
                            ap=[[W_IN, 128], [1, 2], [1, PT]],
                        )
                        nc.tensor.matmul(
                            pp[:, half * PT : (half + 1) * PT], w3, x3,
                            start=True, stop=True,
                            perf_mode=mybir.MatmulPerfMode.DoubleRow,
                        )
                    t0 = 2 * pr * PT
                    if m == 0:
                        # Tile 0: fine 512-col drains prime the drain
                        # pipeline one MM earlier, shortening the ramp
                        # transient (mirror of the last-tile treatment).
                        nc.vector.tensor_copy(
                            out=stage[:, t0 : t0 + PT], in_=pp[:, 0:PT]
                        )
                        nc.scalar.copy(
                            out=stage[:, t0 + PT : t0 + 2 * PT],
                            in_=pp[:, PT : 2 * PT],
                        )
                        if pr == MT // (2 * PT) - 1:
                            nc.gpsimd.dma_start(
                                out=yT[:, YPAD : YPAD + MT],
                                in_=stage[:],
                            )
                    elif not last:
                        # One 1024-col drain per pair (better per-column
                        # rate than 2x512), strictly alternating DVE/ACT.
                        # Re-balancing drains toward the faster ACT engine
                        # was measured WORSE in any placement - the clean
                        # alternating cadence pipelines better than a
                        # nominally balanced split.
                        if pr % 2 == 0:
                            nc.vector.tensor_copy(
                                out=stage[:, t0 : t0 + 2 * PT], in_=pp[:]
                            )
                        else:
                            nc.scalar.copy(
                                out=stage[:, t0 : t0 + 2 * PT], in_=pp[:]
                            )
                        if pr == MT // (2 * PT) - 1:
                            # One whole-tile out-DMA on the GpSimd SWDGE
                            # queue (Q7 emission is ~1us each, so fewer
                            # bigger DMAs; the 6-deep stage ring hides
                            # their completion latency).
                            nc.gpsimd.dma_start(
                                out=yT[
                                    :, YPAD + m * MT : YPAD + (m + 1) * MT
                                ],
                                in_=stage[:],
                            )
                    else:
                        # Last tile: split the pair drain across BOTH
                        # engines and ship each 2048-col half on its own
                        # queue (gpsimd then sync) -> shortest epilogue.
                        nc.vector.tensor_copy(
                            out=stage[:, t0 : t0 + PT], in_=pp[:, 0:PT]
                        )
                        nc.scalar.copy(
                            out=stage[:, t0 + PT : t0 + 2 * PT],
                            in_=pp[:, PT : 2 * PT],
                        )
                        nc.sync.dma_start(
                            out=yT[
                                :,
                                YPAD + m * MT + t0 : YPAD + m * MT + t0
                                + 2 * PT,
                            ],
                            in_=stage[:, t0 : t0 + 2 * PT],
                        )

            # First 128 samples of every row via the dense M0' map.  This
            # plain matmul MUST come after every DoubleRow matmul: a
            # DoubleRow matmul as the final PE instruction wedges the PE
            # (NRT_EXEC_UNIT_UNRECOVERABLE, found by bisection).  Its
            # out-DMA rides the idle Sync queue to keep the epilogue short.
            ps2 = pspool.tile([128, RPC], _F32, name="ps2", tag="ps")
            nc.tensor.matmul(ps2[:], M0, et[:], start=True, stop=True)
            st2 = spool.tile([128, RPC], _F8, tag="st2")
            nc.vector.tensor_copy(out=st2[:], in_=ps2[:])
            nc.sync.dma_start(out=yT[:, YPAD + DCOLS : YCOLS], in_=st2[:])

    nc.compile()  # bacc legalization: <=1 sync wait per HW instruction
    return nc


def _get_nc():
    global _NC_CACHE
    if _NC_CACHE is None:
        _NC_CACHE = _build_nc()
    return _NC_CACHE


# ------------------------------------------------------------- host helpers
def _fir_mat(taps):
    """128x128 matrix of one masked FIR pass: y = T @ x (first 128 samples).

    y[i] = x[i] + sum_j taps[j] * x[i-j-1] for i >= F, else 0.
    """
    T = np.zeros((128, 128))
    for i in range(F, 128):
        T[i, i] = 1.0
        for j in range(F):
            T[i, i - j - 1] += taps[j]
    return T


def _build_weights(h64):
    """Stationary operands, stacked as [C A M0'], fp8."""
    k1 = np.concatenate([[1.0], h64])
    k2 = np.concatenate([[1.0], h64[::-1]])
    kc = np.convolve(k1, k2)  # 33 taps

    i = np.arange(128)
    D = i[None, :] - i[:, None]  # po - p
    A = np.zeros((128, 128))
    mask = (D >= 1) & (D <= 32)  # tail taps only; identity handled on host
    A[mask] = kc[D[mask]]
    Dc = D + 128
    C = np.zeros((128, 128))
    maskc = (Dc >= 1) & (Dc <= 32)
    C[maskc] = kc[Dc[maskc]]

    M0 = (_fir_mat(h64[::-1]) @ _fir_mat(h64)).T  # M0[p, po] = dy[po]/dx[p]
    M0 -= np.eye(128)  # tail-only: host adds x back everywhere

    wmat = np.zeros((128, 384), np.float32)
    wmat[:, 0:128] = C
    wmat[:, 128:256] = A
    wmat[:, 256:384] = M0
    return wmat.astype(_NP8)


def _make_in_maps(x, h64):
    wmat = _build_weights(h64)
    in_maps = []
    for c in range(N_CORES):
        xs = np.ascontiguousarray(x[c * RPC : (c + 1) * RPC])  # (32, 131072)
        Bv = xs.reshape(RPC, BPR, BLK)
        xin = np.zeros((128, 1 + DCOLS), _NP8)
        xin[:, 1:] = Bv.transpose(2, 0, 1).reshape(BLK, DCOLS).astype(_NP8)
        xpk = np.zeros((128, XCOLS), _NP8)
        for m in range(NMT):
            # halo col at slot 15, current cols at slots 16..16+MT-1
            xpk[:, m * W_IN + 15 : (m + 1) * W_IN] = (
                xin[:, m * MT : m * MT + MT + 1]
            )
        xpk[:, NMT * W_IN : XCOLS] = Bv[:, 0, :].astype(_NP8).T
        in_maps.append({"xp": xpk, "wm": wmat})
    return in_maps


def _assemble(x, results):
    y = np.empty((B, N), np.float32)
    for c in range(N_CORES):
        yT = results[c]["yT"].astype(np.float32)
        main = (
            yT[:, YPAD : YPAD + DCOLS]
            .reshape(BLK, RPC, BPR)
            .transpose(1, 2, 0)
            .reshape(RPC, N)
        )
        sl = slice(c * RPC, (c + 1) * RPC)
        y[sl] = x[sl] + main
        y[sl, 0:BLK] = x[sl, 0:BLK] + yT[:, YPAD + DCOLS : YCOLS].T
    y[:, :F] = 0.0
    return y


def _run(x, h, **spmd_kwargs):
    x = np.asarray(x, dtype=np.float32)
    h64 = np.asarray(h, dtype=np.float64).reshape(-1)
    assert x.shape == (B, N) and h64.shape == (F,)
    in_maps = _make_in_maps(x, h64)
    res = run_bass_kernel_spmd(
        _get_nc(), in_maps, core_ids=list(range(N_CORES)), **spmd_kwargs
    )
    return _assemble(x, res.results), res


def kernel(x, kernel):
    y, _ = _run(x, kernel)
    return y


# revision 62
# speedup vs baseline: 1.1001x; 1.0392x over previous
"""Trainium2 Bass kernel for the double-FIR "DeconvLayer" problem.

Reference computation (see problem statement):
    v = mask(conv(x, k1)),  y = mask(conv(v, k2))
with k1 = [1, h], k2 = [1, h_reversed], mask zeroing columns < 16.

Key facts used here:
  * For output column t >= 48 the two masked passes are EXACTLY one causal
    33-tap conv with kc = full_conv(k1, k2):  y[t] = sum_d kc[d] x[t-d].
  * kc[0] = 1, so y = x + e with e = sum_{d>=1} kc[d] x[t-d].  The tail
    term e is small (std ~0.19 vs y std ~1.02), so the DEVICE computes e
    from an fp8 input stream and returns e as fp8; the host assembly adds
    the exactly-kept fp32 x back in.  Wire traffic drops 2x vs fp16 while
    the total rel-l2 error stays ~9e-3 (gate is 2e-2).
  * y[:, 0:128] depends only on x[:, 0:128]: handled exactly (up to fp8)
    by a dense matmul with M0' = M0 - I on separately-staged copies of
    each row's block 0.

Device strategy (pure data parallel, 32 batch rows / core, 8 cores):
  * Host marshals x into time-major columns: each column of the device
    input is one aligned 128-sample block of one row.  Then
        e_block(c) = A.T @ x_block(c) + C.T @ x_block(c-1)
    with A[p, po] = kc[po - p] (in-block taps, d >= 1 only) and
    C[p, po] = kc[po + 128 - p] (halo taps from the previous block).
  * fp8 (e4m3) everywhere on the wire and in the matmuls.  The two
    128-deep contractions (A on block c, C on block c-1) fuse into ONE
    K=256 matmul via MatmulPerfMode.DoubleRow: the stationary operand is
    [128, 2, 128] = [C | A], the moving operand is an overlapping-stride
    AP [128, 2, 512] whose k-tiles are the halo-shifted and in-block
    column windows of the same SBUF data.  One matmul per PSUM tile, one
    weight load for the whole kernel.
  * Engines: DMA (in/out), PE (DoubleRow matmuls), DVE+ACT (PSUM->SBUF
    fp8 casts).  Per-core wire: ~4.2 MB in + ~4.2 MB out, vs the
    ~358 GB/s per-core HBM cap -> ~23.5 us DMA floor.

The harness calls kernel(**inputs) with the FULL inputs; everything the
device needs (shapes, tiling) is hardcoded below.
"""

import numpy as np
import ml_dtypes

import concourse.bass as bass
import concourse.mybir as mybir
from concourse import bacc
from concourse.bass_utils import run_bass_kernel_spmd
from concourse.tile import TileContext

# ---------------------------------------------------------------- geometry
B, N, F = 256, 131072, 16
N_CORES = 8
RPC = B // N_CORES          # 32 batch rows per core
BLK = 128                   # time block = matmul contraction window
BPR = N // BLK              # 1024 blocks per row
DCOLS = RPC * BPR           # 32768 data columns per core
MT = 4096                   # macro-tile: columns per in/out DMA
NMT = DCOLS // MT           # 8 macro tiles per core
PT = 512                    # psum tile columns (one fp32 PSUM bank)
W_IN = MT + 16              # input cols per macro tile (halo + align pad)
XCOLS = NMT * W_IN + RPC    # device input width (+32 row-initial blocks)
YPAD = 32                   # dead lead-in keeps out-DMA rows 32B-aligned
YCOLS = YPAD + DCOLS + RPC  # device output: pad, main, first-block tail

_F8 = mybir.dt.float8e4
_F32 = mybir.dt.float32
_NP8 = ml_dtypes.float8_e4m3

# ------------------------------------------------------------- bass program
_NC_CACHE = None


def _build_nc():
    nc = bacc.Bacc()
    xp = nc.dram_tensor("xp", [128, XCOLS], _F8, kind="ExternalInput")
    wm = nc.dram_tensor("wm", [128, 384], _F8, kind="ExternalInput")
    yT = nc.dram_tensor("yT", [128, YCOLS], _F8, kind="ExternalOutput")

    with TileContext(nc, pool_alloc_mode="queue") as tc:
        with (
            tc.tile_pool(name="w", bufs=1) as wpool,
            tc.tile_pool(name="xin", bufs=5) as xpool,
            tc.tile_pool(name="stage", bufs=6) as spool,
            tc.tile_pool(name="ps", bufs=4, space="PSUM") as pspool,
        ):
            # Weights ride Sync FIRST (49 KB, gates the first LDWEIGHTS),
            # then the input stream owns the queue.  Tile 0 leads with a
            # small chunk so the first matmul starts as early as possible.
            wsb = wpool.tile([128, 384], _F8, tag="wsb")
            nc.sync.dma_start(out=wsb[:], in_=wm[:, :])

            QC = 544   # covers PSUM tile 0 (15 + 512 + halo) + slack
            HC = 2064  # chunk boundary aligned to pair-1's window end
            xt0 = xpool.tile([128, W_IN], _F8, tag="xt")
            nc.sync.dma_start(out=xt0[:, 0:QC], in_=xp[:, 0:QC])
            nc.sync.dma_start(out=xt0[:, QC:HC], in_=xp[:, QC:HC])

            # NOTE: splitting the input stream onto a second DMA path was
            # measured a net loss in every dose: ACT-issued input DMAs
            # wedge the device outright, and SWDGE (gpsimd) input steals
            # HBM bandwidth from the critical early tiles.  Input stays
            # exclusively on the Sync HWDGE ring.
            # DoubleRow stationary operand: ktile0 = C, ktile1 = A.
            w3 = bass.AP(
                tensor=wsb[:].tensor,
                offset=wsb[:].offset,
                ap=[[384, 128], [128, 2], [1, 128]],
            )
            M0 = wsb[:, 256:384]

            # Row-initial blocks for the dense M0' map; prefetched early,
            # consumed by the LAST matmul (see below).
            et = xpool.tile([128, RPC], _F8, tag="extra")
            nc.gpsimd.dma_start(out=et[:], in_=xp[:, NMT * W_IN : XCOLS])

            for m in range(NMT):
                if m == 0:
                    xt = xt0
                    nc.sync.dma_start(
                        out=xt[:, HC:W_IN], in_=xp[:, HC:W_IN]
                    )
                else:
                    # Whole-tile input DMAs: half-splitting the early
                    # tiles (to unlock their first pairs a DMA receipt
                    # earlier) measured neutral-to-worse - the extra
                    # issues occasionally disturb the ramp.
                    xt = xpool.tile([128, W_IN], _F8, tag="xt")
                    w0 = m * W_IN
                    nc.sync.dma_start(
                        out=xt[:], in_=xp[:, w0 : w0 + W_IN]
                    )
                stage = spool.tile([128, MT], _F8, tag="stage")

                # One fused K=256 DoubleRow matmul per 512-col PSUM tile:
                # ktile0 reads the halo-shifted window (col 15+t0+j), ktile1
                # the in-block window (col 16+t0+j) of the same xt bytes.
                # Drain granularity: 1024-col drains are ~25% cheaper per
                # column (fixed instruction overhead), giving DVE/ACT
                # headroom over the PE rate; the LAST tile uses 512-col
                # drains + immediate sync-ring DMAs so the epilogue isn't
                # gated on a coarse drain.
                last = m == NMT - 1
                for pr in range(MT // (2 * PT)):
                    pp = pspool.tile(
                        [128, 2 * PT], _F32, name=f"pp_{m}_{pr}", tag="ps"
                    )
                    for half in range(2):
                        t0 = (2 * pr + half) * PT
                        x3 = bass.AP(
                            tensor=xt[:].tensor,
                            offset=xt[:].offset + 15 + t0,
                            ap=[[W_IN, 128], [1, 2], [1, PT]],
                        )
                        nc.tensor.matmul(
                            pp[:, half * PT : (half + 1) * PT], w3, x3,
                            start=True, stop=True,
                            perf_mode=mybir.MatmulPerfMode.DoubleRow,
                        )
                    t0 = 2 * pr * PT
                    if not last:
                        # One 1024-col drain per pair (better per-column
                        # rate than 2x512), strictly alternating DVE/ACT.
                        # Re-balancing drains toward the faster ACT engine
                        # was measured WORSE in any placement - the clean
                        # alternating cadence pipelines better than a
                        # nominally balanced split.
                        if pr % 2 == 0:
                            nc.vector.tensor_copy(
                                out=stage[:, t0 : t0 + 2 * PT], in_=pp[:]
                            )
                        else:
                            nc.scalar.copy(
                                out=stage[:, t0 : t0 + 2 * PT], in_=pp[:]
                            )
                        if pr == MT // (2 * PT) - 1:
                            # One whole-tile out-DMA on the GpSimd SWDGE
                            # queue (Q7 emission is ~1us each, so fewer
                            # bigger DMAs; the 6-deep stage ring hides
                            # their completion latency).
                            nc.gpsimd.dma_start(
                                out=yT[
                                    :, YPAD + m * MT : YPAD + (m + 1) * MT
                                ],
                                in_=stage[:],
                            )
                    else:
                        # Last tile: split the pair drain across BOTH
                        # engines and ship each 2048-col half on its own
                        # queue (gpsimd then sync) -> shortest epilogue.
                        nc.vector.tensor_copy(
                            out=stage[:, t0 : t0 + PT], in_=pp[:, 0:PT]
                        )
                        nc.scalar.copy(
                            out=stage[:, t0 + PT : t0 + 2 * PT],
                            in_=pp[:, PT : 2 * PT],
                        )
                        nc.sync.dma_start(
                            out=yT[
                                :,
                                YPAD + m * MT + t0 : YPAD + m * MT + t0
                                + 2 * PT,
                            ],
                            in_=stage[:, t0 : t0 + 2 * PT],
                        )

            # First 128 samples of every row via the dense M0' map.  This
            # plain matmul MUST come after every DoubleRow matmul: a
            # DoubleRow matmul as the final PE instruction wedges the PE
            # (NRT_EXEC_UNIT_UNRECOVERABLE, found by bisection).  Its
            # out-DMA rides the idle Sync queue to keep the epilogue short.
            ps2 = pspool.tile([128, RPC], _F32, name="ps2", tag="ps")
            nc.tensor.matmul(ps2[:], M0, et[:], start=True, stop=True)
            st2 = spool.tile([128, RPC], _F8, tag="st2")
            nc.vector.tensor_copy(out=st2[:], in_=ps2[:])
            nc.sync.dma_start(out=yT[:, YPAD + DCOLS : YCOLS], in_=st2[:])

    nc.compile()  # bacc legalization: <=1 sync wait per HW instruction
    return nc


def _get_nc():
    global _NC_CACHE
    if _NC_CACHE is None:
        _NC_CACHE = _build_nc()
    return _NC_CACHE


# ------------------------------------------------------------- host helpers
def _fir_mat(taps):
    """128x128 matrix of one masked FIR pass: y = T @ x (first 128 samples).

    y[i] = x[i] + sum_j taps[j] * x[i-j-1] for i >= F, else 0.
    """
    T = np.zeros((128, 128))
    for i in range(F, 128):
        T[i, i] = 1.0
        for j in range(F):
            T[i, i - j - 1] += taps[j]
    return T


def _build_weights(h64):
    """Stationary operands, stacked as [C A M0'], fp8."""
    k1 = np.concatenate([[1.0], h64])
    k2 = np.concatenate([[1.0], h64[::-1]])
    kc = np.convolve(k1, k2)  # 33 taps

    i = np.arange(128)
    D = i[None, :] - i[:, None]  # po - p
    A = np.zeros((128, 128))
    mask = (D >= 1) & (D <= 32)  # tail taps only; identity handled on host
    A[mask] = kc[D[mask]]
    Dc = D + 128
    C = np.zeros((128, 128))
    maskc = (Dc >= 1) & (Dc <= 32)
    C[maskc] = kc[Dc[maskc]]

    M0 = (_fir_mat(h64[::-1]) @ _fir_mat(h64)).T  # M0[p, po] = dy[po]/dx[p]
    M0 -= np.eye(128)  # tail-only: host adds x back everywhere

    wmat = np.zeros((128, 384), np.float32)
    wmat[:, 0:128] = C
    wmat[:, 128:256] = A
    wmat[:, 256:384] = M0
    return wmat.astype(_NP8)


def _make_in_maps(x, h64):
    wmat = _build_weights(h64)
    in_maps = []
    for c in range(N_CORES):
        xs = np.ascontiguousarray(x[c * RPC : (c + 1) * RPC])  # (32, 131072)
        Bv = xs.reshape(RPC, BPR, BLK)
        xin = np.zeros((128, 1 + DCOLS), _NP8)
        xin[:, 1:] = Bv.transpose(2, 0, 1).reshape(BLK, DCOLS).astype(_NP8)
        xpk = np.zeros((128, XCOLS), _NP8)
        for m in range(NMT):
            # halo col at slot 15, current cols at slots 16..16+MT-1
            xpk[:, m * W_IN + 15 : (m + 1) * W_IN] = (
                xin[:, m * MT : m * MT + MT + 1]
            )
        xpk[:, NMT * W_IN : XCOLS] = Bv[:, 0, :].astype(_NP8).T
        in_maps.append({"xp": xpk, "wm": wmat})
    return in_maps


def _assemble(x, results):
    y = np.empty((B, N), np.float32)
    for c in range(N_CORES):
        yT = results[c]["yT"].astype(np.float32)
        main = (
            yT[:, YPAD : YPAD + DCOLS]
            .reshape(BLK, RPC, BPR)
            .transpose(1, 2, 0)
            .reshape(RPC, N)
        )
        sl = slice(c * RPC, (c + 1) * RPC)
        y[sl] = x[sl] + main
        y[sl, 0:BLK] = x[sl, 0:BLK] + yT[:, YPAD + DCOLS : YCOLS].T
    y[:, :F] = 0.0
    return y


def _run(x, h, **spmd_kwargs):
    x = np.asarray(x, dtype=np.float32)
    h64 = np.asarray(h, dtype=np.float64).reshape(-1)
    assert x.shape == (B, N) and h64.shape == (F,)
    in_maps = _make_in_maps(x, h64)
    res = run_bass_kernel_spmd(
        _get_nc(), in_maps, core_ids=list(range(N_CORES)), **spmd_kwargs
    )
    return _assemble(x, res.results), res


def kernel(x, kernel):
    y, _ = _run(x, kernel)
    return y
